# revision 6
# baseline (speedup 1.0000x reference)
"""GCNConv Trainium2 kernel: 8-core SPMD via bass/Tile (v3).

Strategy (dst-range edge sharding; one shared SPMD program, all data per-core):
  - core c owns dst nodes [c*NSH, (c+1)*NSH) and all edges into them
  - table xd = (dis * feat) @ fc_w.T built on device in bf16 (dis = deg^-0.5
    folded into the table; edge_b folded into row 7 of the edge-weight matmul)
  - 4 bucket passes over src space (25088 nodes each): bucket table loaded
    into SBUF TRANSPOSED [128 feat, nodes] via dma_start_transpose, then
    per (sw, group) run: ap_gather (GPSIMD compute gather along free dim)
    pulls src columns -> xdT [128f, e]; pwT = ewT9 @ efd (stationary weights,
    N=512 batches); mpreT = xdT + pwT (DVE, bf16); PE transpose -> PSUM;
    m = relu (ACT, PSUM->SBUF bf16); segment-sum matmul with host-built
    one-hot (streamed from DRAM bf16) into PSUM h^T at static offsets
  - node side: out = hT*dis + relu(xd/dis + root)/deg via PE transpose
"""
import sys, math
sys.path.insert(0, "/opt/trn_rl_repo")
import numpy as np

from concourse import bass, bacc, mybir, tile
from concourse import bass_utils

f32 = mybir.dt.float32
bf16 = mybir.dt.bfloat16
i16 = mybir.dt.int16
RELU = mybir.ActivationFunctionType.Relu
ALU = mybir.AluOpType


class Cfg:
    def __init__(self, N=100000, E=1600000, F=128, ED=7, cores=8,
                 grp=512, cap_full=19, cap_last=9):
        self.N, self.E, self.F, self.ED, self.cores = N, E, F, ED, cores
        self.NSH = N // cores                    # 12500 nodes per core
        self.GRP = grp
        self.SW = 2048                           # psum super-window (4 banks)
        self.n_groups = math.ceil(self.NSH / grp)           # 25
        self.gps = self.SW // grp                # groups per full SW (4)
        self.n_sw = math.ceil(self.n_groups / self.gps)     # 7
        self.last_w = self.NSH - (self.n_groups - 1) * grp  # 212
        self.cap_full, self.cap_last = cap_full, cap_last
        self.n_buckets = 4
        self.bucket_sz = 25088                   # 196 tiles of 128
        self.btiles = self.bucket_sz // 128      # 196
        self.Npad = self.n_buckets * self.bucket_sz         # 100352
        self.cap = [cap_full] * (self.n_groups - 1) + [cap_last]
        self.chunks_per_b = sum(self.cap)        # 465
        self.n_chunks = self.n_buckets * self.chunks_per_b  # 1860
        self.slots = self.n_chunks * 128
        self.slots_b = self.chunks_per_b * 128   # 59520
        self.nsh_tiles = math.ceil(self.NSH / 128)
        self.NSHpad = self.nsh_tiles * 128

    def groups_of_sw(self, s):
        g0 = s * self.gps
        return list(range(g0, min(g0 + self.gps, self.n_groups)))

    def sched(self):
        """Static chunk schedule: (bucket, sw, group, start, stop) per chunk.
        Bucket-major: for b, for sw, for group in sw, cap chunks."""
        out = []
        for b in range(self.n_buckets):
            for s in range(self.n_sw):
                for g in self.groups_of_sw(s):
                    for k in range(self.cap[g]):
                        out.append((b, s, g, k == 0, k == self.cap[g] - 1))
        assert len(out) == self.n_chunks
        return out


CFG = Cfg()
_PROG_CACHE = {}


# ---------------------------------------------------------------- program ----
def build_program(cfg: Cfg):
    nc = bacc.Bacc("TRN2", target_bir_lowering=False, debug=False,
                   num_devices=cfg.cores)
    F, GRP, SW = cfg.F, cfg.GRP, cfg.SW

    featT_d = nc.dram_tensor("featT", [F, cfg.Npad], bf16, kind="ExternalInput")
    fcwT_d = nc.dram_tensor("fcwT", [F, F], bf16, kind="ExternalInput")
    ewT9_d = nc.dram_tensor("ewT9", [8, F], bf16, kind="ExternalInput")
    rootB_d = nc.dram_tensor("rootB", [128, F], f32, kind="ExternalInput")
    ident_d = nc.dram_tensor("ident", [128, 128], f32, kind="ExternalInput")
    identB_d = nc.dram_tensor("identB", [128, 128], bf16, kind="ExternalInput")
    efT_d = nc.dram_tensor("efT", [8, cfg.slots], bf16, kind="ExternalInput")
    oh_d = nc.dram_tensor("ohT", [128, cfg.n_chunks * GRP], bf16,
                          kind="ExternalInput")
    idx_d = nc.dram_tensor("idxw", [128, cfg.slots // 16], i16,
                           kind="ExternalInput")
    disP_d = nc.dram_tensor("disP", [128, cfg.nsh_tiles], f32,
                            kind="ExternalInput")
    ivdP_d = nc.dram_tensor("ivdP", [128, cfg.nsh_tiles], f32,
                            kind="ExternalInput")
    idisP_d = nc.dram_tensor("idisP", [128, cfg.nsh_tiles], f32,
                             kind="ExternalInput")

    xb_d = [nc.dram_tensor(f"xb{b}", [cfg.bucket_sz, F], bf16, kind="Internal")
            for b in range(cfg.n_buckets)]
    out_d = nc.dram_tensor("out", [cfg.NSHpad, F], f32, kind="ExternalOutput")

    with tile.TileContext(nc) as tc:
        with tc.tile_pool(name="persist", bufs=1) as pers:
            fcwT = pers.tile([F, F], bf16)
            nc.sync.dma_start(out=fcwT[:], in_=fcwT_d.ap())
            ewT9 = pers.tile([8, F], bf16)
            nc.sync.dma_start(out=ewT9[:], in_=ewT9_d.ap())
            rootB = pers.tile([128, F], f32)
            nc.sync.dma_start(out=rootB[:], in_=rootB_d.ap())
            ident = pers.tile([128, 128], f32)
            nc.sync.dma_start(out=ident[:], in_=ident_d.ap())
            identB = pers.tile([128, 128], bf16)
            nc.sync.dma_start(out=identB[:], in_=identB_d.ap())
            disP = pers.tile([128, cfg.nsh_tiles], f32)
            nc.sync.dma_start(out=disP[:], in_=disP_d.ap())
            ivdP = pers.tile([128, cfg.nsh_tiles], f32)
            nc.sync.dma_start(out=ivdP[:], in_=ivdP_d.ap())
            idisP = pers.tile([128, cfg.nsh_tiles], f32)
            nc.sync.dma_start(out=idisP[:], in_=idisP_d.ap())
            hT = pers.tile([128, cfg.NSHpad], f32)   # h^T [feat, node]
            nc.vector.memset(hT[:], 0.0)

            # ================= phase 1: xd table (per bucket) =================
            with (
                tc.tile_pool(name="xph", bufs=3) as xph,
                tc.tile_pool(name="xps", bufs=4, space="PSUM") as xps,
            ):
                BLK = 4
                for b in range(cfg.n_buckets):
                    for blk in range(cfg.btiles // BLK):   # 49 blocks
                        t0 = b * cfg.btiles + blk * BLK
                        ft = xph.tile([F, BLK * 128], bf16, tag="ft")
                        nc.sync.dma_start(
                            out=ft[:],
                            in_=featT_d.ap()[:, t0 * 128:(t0 + BLK) * 128])
                        px = xps.tile([128, BLK, F], f32, tag="px")
                        for j in range(BLK):
                            nc.tensor.matmul(
                                out=px[:, j, :],
                                lhsT=ft[:, j * 128:(j + 1) * 128],
                                rhs=fcwT[:], start=True, stop=True)
                        xt = xph.tile([128, BLK, F], bf16, tag="xt")
                        nc.vector.tensor_copy(out=xt[:], in_=px[:])
                        nc.sync.dma_start(
                            out=xb_d[b].ap()[blk * BLK * 128:
                                             (blk + 1) * BLK * 128, :]
                            .rearrange("(b p) f -> p b f", p=128),
                            in_=xt[:])

            # ================= phase 2: edges (bucket passes) =================
            sched = cfg.sched()
            with (
                tc.tile_pool(name="tabp", bufs=1) as tabp,
                tc.tile_pool(name="idxp", bufs=2) as idxp,
                tc.tile_pool(name="eph", bufs=2) as eph,
                tc.tile_pool(name="mph", bufs=3) as mph,
                tc.tile_pool(name="mmp", bufs=4) as mmp,
                tc.tile_pool(name="hps_pool", bufs=1, space="PSUM") as hps_pool,
                tc.tile_pool(name="wps_pool", bufs=2, space="PSUM") as wps_pool,
                tc.tile_pool(name="tps_pool", bufs=2, space="PSUM") as tps_pool,
            ):
                hps = hps_pool.tile([128, SW], f32)
                ci = 0
                for b in range(cfg.n_buckets):
                    tabT = tabp.tile([128, cfg.bucket_sz], bf16, tag="tabT")
                    nc.sync.dma_start_transpose(out=tabT[:], in_=xb_d[b].ap())
                    idxb = idxp.tile([128, cfg.slots_b // 16], i16, tag="idxb")
                    nc.sync.dma_start(
                        out=idxb[:],
                        in_=idx_d.ap()[:, b * (cfg.slots_b // 16):
                                       (b + 1) * (cfg.slots_b // 16)])
                    sib = 0            # slot index within bucket
                    for s in range(cfg.n_sw):
                        for g in cfg.groups_of_sw(s):
                            cap = cfg.cap[g]
                            nidx = cap * 128
                            off = (g - s * cfg.gps) * GRP
                            # pair-gather: idx = src//2, d=2 -> [128, j, 2];
                            # slot parity matches src parity (host-packed)
                            gx = eph.tile([128, cfg.cap_full * 64, 2, 2],
                                          bf16, tag="gx")
                            nc.gpsimd.ap_gather(
                                out_ap=gx[:, :cap * 64, :, :],
                                in_ap=tabT[:, :],
                                idxs_ap=idxb[:, sib // 16:(sib + nidx) // 16],
                                channels=128, num_elems=cfg.bucket_sz // 2,
                                d=2, num_idxs=nidx)
                            ef = eph.tile([8, cfg.cap_full * 128], bf16,
                                          tag="ef")
                            gsi = b * cfg.slots_b + sib
                            nc.sync.dma_start(
                                out=ef[:, :nidx],
                                in_=efT_d.ap()[:, gsi:gsi + nidx])
                            oh = eph.tile([128, cfg.cap_full * GRP], bf16,
                                          tag="oh")
                            nc.sync.dma_start(
                                out=oh[:, :cap * GRP],
                                in_=oh_d.ap()[:, ci * GRP:(ci + cap) * GRP])
                            for q in range(0, cap, 4):
                                bs = min(4, cap - q)
                                nj = bs * 64
                                pwT = wps_pool.tile([128, 256, 2], f32,
                                                    tag="pwT")
                                nc.tensor.matmul(
                                    out=pwT[:, :nj, :],
                                    lhsT=ewT9[:],
                                    rhs=ef[:, q * 128:q * 128 + bs * 128],
                                    start=True, stop=True)
                                mpreT = mph.tile([128, 256, 2], bf16,
                                                 tag="mpreT")
                                j0 = q * 64
                                nc.vector.tensor_add(
                                    out=mpreT[:, :nj, 0],
                                    in0=gx[:, j0:j0 + nj, 0, 0],
                                    in1=pwT[:, :nj, 0])
                                nc.vector.tensor_add(
                                    out=mpreT[:, :nj, 1],
                                    in0=gx[:, j0:j0 + nj, 1, 1],
                                    in1=pwT[:, :nj, 1])
                                for kk in range(bs):
                                    _b, _s, _g, st, sp = sched[ci]
                                    assert (_b, _s, _g) == (b, s, g)
                                    tp = tps_pool.tile([128, 128], bf16,
                                                       tag="tp")
                                    nc.tensor.transpose(
                                        out=tp[:],
                                        in_=mpreT[:, kk * 64:(kk + 1) * 64, :],
                                        identity=identB[:])
                                    m = mmp.tile([128, F], bf16, tag="m")
                                    nc.scalar.activation(
                                        out=m[:], in_=tp[:], func=RELU)
                                    nc.tensor.matmul(
                                        out=hps[:, off:off + GRP],
                                        lhsT=m[:],
                                        rhs=oh[:, (q + kk) * GRP:
                                               (q + kk + 1) * GRP],
                                        start=st, stop=sp,
                                        skip_group_check=True)
                                    ci += 1
                            sib += nidx
                        w = SW if s < cfg.n_sw - 1 else cfg.last_w
                        nc.vector.tensor_add(
                            out=hT[:, s * SW:s * SW + w],
                            in0=hT[:, s * SW:s * SW + w], in1=hps[:, :w])
                assert ci == cfg.n_chunks

            # ================= phase 3: node-side =================
            with (
                tc.tile_pool(name="nph", bufs=3) as nph,
                tc.tile_pool(name="nps", bufs=4, space="PSUM") as nps,
            ):
                NBLK = 8
                for blk in range(math.ceil(cfg.nsh_tiles / NBLK)):
                    t0 = blk * NBLK
                    nt = min(NBLK, cfg.nsh_tiles - t0)
                    xtile = nph.tile([128, NBLK, F], bf16, tag="xtile")
                    nc.sync.dma_start(
                        out=xtile[:, :nt, :],
                        in_=xb_d[0].ap()[t0 * 128:(t0 + nt) * 128, :].rearrange(
                            "(b p) f -> p b f", p=128))
                    ot = nph.tile([128, NBLK, F], f32, tag="ot")
                    for j in range(nt):
                        t = t0 + j
                        pt = nps.tile([128, F], f32, tag="pt")
                        nc.tensor.transpose(
                            out=pt[:], in_=hT[:, t * 128:(t + 1) * 128],
                            identity=ident[:])
                        s1 = nph.tile([128, F], f32, tag="s1")
                        nc.vector.tensor_scalar_mul(
                            out=s1[:], in0=pt[:], scalar1=disP[:, t:t + 1])
                        x1 = nph.tile([128, F], f32, tag="x1")
                        nc.vector.tensor_scalar_mul(
                            out=x1[:], in0=xtile[:, j, :],
                            scalar1=idisP[:, t:t + 1])
                        t1 = nph.tile([128, F], f32, tag="t1")
                        nc.vector.tensor_add(
                            out=t1[:], in0=x1[:], in1=rootB[:])
                        s2 = nph.tile([128, F], f32, tag="s2")
                        nc.scalar.activation(
                            out=s2[:], in_=t1[:], func=RELU,
                            scale=ivdP[:, t:t + 1])
                        nc.vector.tensor_add(out=ot[:, j, :], in0=s1[:],
                                             in1=s2[:])
                    nc.sync.dma_start(
                        out=out_d.ap()[t0 * 128:(t0 + nt) * 128, :].rearrange(
                            "(b p) f -> p b f", p=128),
                        in_=ot[:, :nt, :])
    nc.compile()
    return nc


# ------------------------------------------------------------- host prep ----
def host_prep(cfg: Cfg, feat, edge_feat, src, dst, fc_w, edge_w, edge_b,
              root_emb):
    import ml_dtypes
    bf = ml_dtypes.bfloat16
    N, E, F = cfg.N, cfg.E, cfg.F
    deg = (np.bincount(dst, minlength=N) + 1.0).astype(np.float32)
    dis = deg ** -0.5

    featT_full = np.ascontiguousarray((feat * dis[:, None]).T).astype(np.float32)
    fcwT = np.ascontiguousarray(fc_w.T).astype(bf)
    ewT9 = np.zeros((8, F), dtype=np.float32)
    ewT9[:cfg.ED] = edge_w.T
    ewT9[7] = edge_b
    ewT9 = ewT9.astype(bf)
    rootB = np.tile(root_emb[0][None, :], (128, 1)).astype(np.float32)
    ident = np.eye(128, dtype=np.float32)

    core_of = dst // cfg.NSH
    in_maps = []
    for c in range(cfg.cores):
        sel = np.nonzero(core_of == c)[0]
        rsrc = (src[sel] - c * cfg.NSH) % N       # rotated table space
        ed = dst[sel] - c * cfg.NSH
        eb = rsrc // cfg.bucket_sz
        g = ed // cfg.GRP
        s_of_g = np.minimum(g // cfg.gps, cfg.n_sw - 1)
        # schedule-order composite key: (bucket, sw, group)
        comp = (eb * cfg.n_sw + s_of_g) * cfg.n_groups + g
        order = np.lexsort((ed, comp))
        es, ed, eb, g, comp = (rsrc[order], ed[order], eb[order], g[order],
                               comp[order])
        eid = sel[order]

        slot_src = np.zeros(cfg.slots, dtype=np.int16)
        slot_rel = np.full(cfg.slots, -1, dtype=np.int64)
        slot_eid = np.full(cfg.slots, -1, dtype=np.int64)

        keys = [(b * cfg.n_sw + s) * cfg.n_groups + gg
                for b in range(cfg.n_buckets) for s in range(cfg.n_sw)
                for gg in cfg.groups_of_sw(s)]
        seg_starts = np.searchsorted(comp, keys + [keys[-1] + 1])
        ci = 0
        seg = 0
        for b in range(cfg.n_buckets):
            for s in range(cfg.n_sw):
                for gg in cfg.groups_of_sw(s):
                    lo, hi = seg_starts[seg], seg_starts[seg + 1]
                    seg += 1
                    seg_es, seg_ed, seg_eid = es[lo:hi], ed[lo:hi], eid[lo:hi]
                    slot0 = ci * 128
                    # parity packing: even-src edges -> even slots, odd -> odd
                    for par in (0, 1):
                        psel = np.nonzero((seg_es & 1) == par)[0]
                        npar = len(psel)
                        if math.ceil(npar / 64) > cfg.cap[gg]:
                            raise RuntimeError(
                                f"overflow core {c} b {b} sw {s} g {gg} "
                                f"par {par}: {math.ceil(npar/64)} > "
                                f"{cfg.cap[gg]}")
                        jj = np.arange(npar)
                        slots = (slot0 + (jj // 64) * 128 +
                                 (jj % 64) * 2 + par)
                        slot_src[slots] = ((seg_es[psel] - b * cfg.bucket_sz)
                                           >> 1).astype(np.int16)
                        slot_rel[slots] = seg_ed[psel] - gg * cfg.GRP
                        slot_eid[slots] = seg_eid[psel]
                    ci += cfg.cap[gg]
        assert ci == cfg.n_chunks

        real = slot_eid >= 0
        efT = np.zeros((8, cfg.slots), dtype=bf)
        sdis = dis[src[slot_eid[real]]].astype(np.float32)
        efT[:cfg.ED, real] = (edge_feat[slot_eid[real]] *
                              sdis[:, None]).T.astype(bf)
        efT[7, real] = sdis.astype(bf)

        oh = np.zeros((128, cfg.n_chunks * cfg.GRP), dtype=bf)
        slots_idx = np.nonzero(real)[0]
        rows = slots_idx % 128
        cols = (slots_idx // 128) * cfg.GRP + slot_rel[slots_idx]
        oh[rows, cols] = 1.0

        # idx wrap: within each (b,s,g) call window, idx j at [j%16, j//16]
        idxw = np.zeros((16, cfg.slots // 16), dtype=np.int16)
        si = 0
        for b in range(cfg.n_buckets):
            for s in range(cfg.n_sw):
                for gg in cfg.groups_of_sw(s):
                    nch = cfg.cap[gg]
                    blkv = slot_src[si:si + nch * 128]
                    idxw[:, si // 16:(si + nch * 128) // 16] = \
                        blkv.reshape(-1, 16).T
                    si += nch * 128
        idxw = np.tile(idxw, (8, 1))

        nd = np.arange(cfg.NSHpad)
        gidx = np.minimum(c * cfg.NSH + nd, N - 1)
        disP = np.ascontiguousarray(dis[gidx].reshape(-1, 128).T)
        ivdP = np.ascontiguousarray((1.0 / deg[gidx]).reshape(-1, 128).T)
        idisP = np.ascontiguousarray((1.0 / dis[gidx]).reshape(-1, 128).T)

        featT = np.zeros((F, cfg.Npad), dtype=np.float32)
        featT[:, :N] = np.roll(featT_full, -c * cfg.NSH, axis=1)

        in_maps.append({
            "featT": featT.astype(bf), "fcwT": fcwT, "ewT9": ewT9,
            "rootB": rootB, "ident": ident, "identB": ident.astype(bf),
            "efT": efT, "ohT": oh, "idxw": idxw,
            "disP": disP, "ivdP": ivdP, "idisP": idisP,
        })
    return in_maps


# ----------------------------------------------------------------- entry ----
def kernel(feat, edge_feat, src, dst, fc_w, edge_w, edge_b, root_emb,
           _trace=False, _cfg=None, **_kw):
    cfg = _cfg or CFG
    feat = np.asarray(feat); edge_feat = np.asarray(edge_feat)
    src = np.asarray(src); dst = np.asarray(dst)
    fc_w = np.asarray(fc_w); edge_w = np.asarray(edge_w)
    edge_b = np.asarray(edge_b); root_emb = np.asarray(root_emb)
    assert feat.shape == (cfg.N, cfg.F) and src.shape == (cfg.E,), \
        (feat.shape, src.shape)
    key = id(cfg) if _cfg is not None else "main"
    if key not in _PROG_CACHE:
        _PROG_CACHE[key] = build_program(cfg)
    nc = _PROG_CACHE[key]
    in_maps = host_prep(
        cfg, feat, edge_feat, src, dst, fc_w, edge_w, edge_b, root_emb)
    res = bass_utils.run_bass_kernel_spmd(
        nc, in_maps, core_ids=list(range(cfg.cores)), trace=_trace)
    out = np.concatenate(
        [res.results[c]["out"][:cfg.NSH] for c in range(cfg.cores)], axis=0)
    kernel._last_results = res
    return out.astype(np.float32)


# revision 7
# speedup vs baseline: 3.2610x; 3.2610x over previous
"""GCNConv Trainium2 kernel: 8-core SPMD via bass/Tile (v4).

Strategy (dst-range edge sharding; one shared SPMD program, all data per-core):
  - core c owns dst nodes [c*NSH, (c+1)*NSH) and all edges into them
  - table xd = (dis * feat) @ fc_w.T built on device in bf16 (dis = deg^-0.5
    folded into the table; edge_b folded into row 7 of the edge-weight matmul)
  - per-edge chunks (128 edges within one (512-dst-group, src-bucket) cell,
    fully static schedule with data-exact per-cell caps): dma_gather xd[src]
    (256B bf16 rows), pw = efd @ ewT9 on PE, mpre = gather + pw (DVE, batch
    4 chunks), m = relu(mpre) (ACT, bf16, batch 4), segment-sum matmul with
    host-precomputed one-hot (streamed from DRAM, bf16) into PSUM h^T
    [feat, 2048-node super-window] at static column offsets
  - node side: out = hT*dis + relu(xd/dis + root)/deg via PE transpose
"""
import sys, math
sys.path.insert(0, "/opt/trn_rl_repo")
import numpy as np

from concourse import bass, bacc, mybir, tile
from concourse import bass_utils

f32 = mybir.dt.float32
bf16 = mybir.dt.bfloat16
i16 = mybir.dt.int16
RELU = mybir.ActivationFunctionType.Relu
ALU = mybir.AluOpType


class Cfg:
    def __init__(self, N=100000, E=1600000, F=128, ED=7, cores=8,
                 grp=512, gb=26):
        self.N, self.E, self.F, self.ED, self.cores = N, E, F, ED, cores
        self.NSH = N // cores                    # 12500 nodes per core
        self.GRP = grp
        self.SW = 2048                           # psum super-window (4 banks)
        self.n_groups = math.ceil(self.NSH / grp)           # 25
        self.gps = self.SW // grp                # groups per full SW (4)
        self.n_sw = math.ceil(self.n_groups / self.gps)     # 7
        self.last_w = self.NSH - (self.n_groups - 1) * grp  # 212
        self.n_buckets = 4
        self.bucket_sz = 25088
        self.btiles = self.bucket_sz // 128      # 196
        self.Npad = self.n_buckets * self.bucket_sz         # 100352
        self.gb = gb                             # max chunks per gather call
        self.nsh_tiles = math.ceil(self.NSH / 128)
        self.NSHpad = self.nsh_tiles * 128
        self.caps = None                         # [n_buckets, n_groups]

    def set_caps(self, caps):
        caps = np.asarray(caps, dtype=np.int64).copy()
        caps[0] = np.maximum(caps[0], 1)   # ensure a start=True chunk per grp
        self.caps = caps
        self.n_chunks = int(caps.sum())
        self.slots = self.n_chunks * 128

    def groups_of_sw(self, s):
        g0 = s * self.gps
        return list(range(g0, min(g0 + self.gps, self.n_groups)))

    def sched(self):
        """Static chunk schedule: (sw, bucket, group, start, stop).
        Order: for sw, for bucket, for group in sw, caps[b,g] chunks."""
        first = {}
        last = {}
        order = []
        for s in range(self.n_sw):
            for b in range(self.n_buckets):
                for g in self.groups_of_sw(s):
                    for k in range(int(self.caps[b, g])):
                        if (s, g) not in first:
                            first[(s, g)] = len(order)
                        last[(s, g)] = len(order)
                        order.append([s, b, g, False, False])
        for key, i in first.items():
            order[i][3] = True
        for key, i in last.items():
            order[i][4] = True
        assert len(order) == self.n_chunks
        return [tuple(x) for x in order]

    def call_layout(self):
        """Per (sw, bucket): list of gather-call chunk counts."""
        out = []
        for s in range(self.n_sw):
            for b in range(self.n_buckets):
                nch = int(sum(self.caps[b, g] for g in self.groups_of_sw(s)))
                rem, sizes = nch, []
                while rem > 0:
                    sizes.append(min(self.gb, rem))
                    rem -= sizes[-1]
                out.append(sizes)
        return out


CFG = Cfg()
_PROG_CACHE = {}


# ---------------------------------------------------------------- program ----
def build_program(cfg: Cfg):
    nc = bacc.Bacc("TRN2", target_bir_lowering=False, debug=False,
                   num_devices=cfg.cores)
    F, GRP, SW = cfg.F, cfg.GRP, cfg.SW

    featT_d = nc.dram_tensor("featT", [F, cfg.Npad], bf16, kind="ExternalInput")
    fcwT_d = nc.dram_tensor("fcwT", [F, F], bf16, kind="ExternalInput")
    ewT9_d = nc.dram_tensor("ewT9", [8, F], bf16, kind="ExternalInput")
    rootB_d = nc.dram_tensor("rootB", [128, F], f32, kind="ExternalInput")
    ident_d = nc.dram_tensor("ident", [128, 128], f32, kind="ExternalInput")
    efT_d = nc.dram_tensor("efT", [8, cfg.slots], bf16, kind="ExternalInput")
    oh_d = nc.dram_tensor("ohT", [128, cfg.n_chunks * GRP], bf16,
                          kind="ExternalInput")
    idx_d = nc.dram_tensor("idxw", [128, cfg.slots // 16], i16,
                           kind="ExternalInput")
    disP_d = nc.dram_tensor("disP", [128, cfg.nsh_tiles], f32,
                            kind="ExternalInput")
    ivdP_d = nc.dram_tensor("ivdP", [128, cfg.nsh_tiles], f32,
                            kind="ExternalInput")
    idisP_d = nc.dram_tensor("idisP", [128, cfg.nsh_tiles], f32,
                             kind="ExternalInput")

    xb_d = [nc.dram_tensor(f"xb{b}", [cfg.bucket_sz, F], bf16, kind="Internal")
            for b in range(cfg.n_buckets)]
    out_d = nc.dram_tensor("out", [cfg.NSHpad, F], f32, kind="ExternalOutput")

    with tile.TileContext(nc) as tc:
        with tc.tile_pool(name="persist", bufs=1) as pers:
            fcwT = pers.tile([F, F], bf16)
            nc.sync.dma_start(out=fcwT[:], in_=fcwT_d.ap())
            ewT9 = pers.tile([8, F], bf16)
            nc.sync.dma_start(out=ewT9[:], in_=ewT9_d.ap())
            rootB = pers.tile([128, F], f32)
            nc.sync.dma_start(out=rootB[:], in_=rootB_d.ap())
            ident = pers.tile([128, 128], f32)
            nc.sync.dma_start(out=ident[:], in_=ident_d.ap())
            idxw = pers.tile([128, cfg.slots // 16], i16)
            nc.sync.dma_start(out=idxw[:], in_=idx_d.ap())
            disP = pers.tile([128, cfg.nsh_tiles], f32)
            nc.sync.dma_start(out=disP[:], in_=disP_d.ap())
            ivdP = pers.tile([128, cfg.nsh_tiles], f32)
            nc.sync.dma_start(out=ivdP[:], in_=ivdP_d.ap())
            idisP = pers.tile([128, cfg.nsh_tiles], f32)
            nc.sync.dma_start(out=idisP[:], in_=idisP_d.ap())
            hT = pers.tile([128, cfg.NSHpad], f32)   # h^T [feat, node]
            nc.vector.memset(hT[:], 0.0)

            # ================= phase 1: xd table (per bucket) =================
            with (
                tc.tile_pool(name="xph", bufs=3) as xph,
                tc.tile_pool(name="xps", bufs=3, space="PSUM") as xps,
            ):
                BLK = 4
                for b in range(cfg.n_buckets):
                    for blk in range(cfg.btiles // BLK):   # 49 blocks
                        t0 = b * cfg.btiles + blk * BLK
                        ft = xph.tile([F, BLK * 128], bf16, tag="ft")
                        nc.sync.dma_start(
                            out=ft[:],
                            in_=featT_d.ap()[:, t0 * 128:(t0 + BLK) * 128])
                        px = xps.tile([128, BLK, F], f32, tag="px")
                        for j in range(BLK):
                            nc.tensor.matmul(
                                out=px[:, j, :],
                                lhsT=ft[:, j * 128:(j + 1) * 128],
                                rhs=fcwT[:], start=True, stop=True)
                        xt = xph.tile([128, BLK, F], bf16, tag="xt")
                        nc.vector.tensor_copy(out=xt[:], in_=px[:])
                        nc.sync.dma_start(
                            out=xb_d[b].ap()[blk * BLK * 128:
                                             (blk + 1) * BLK * 128, :]
                            .rearrange("(b p) f -> p b f", p=128),
                            in_=xt[:])

            # ================= phase 2: edges =================
            sched = cfg.sched()
            calls = cfg.call_layout()
            with (
                tc.tile_pool(name="eph", bufs=2) as eph,
                tc.tile_pool(name="mph", bufs=3) as mph,
                tc.tile_pool(name="hps_pool", bufs=1, space="PSUM") as hps_pool,
                tc.tile_pool(name="wps_pool", bufs=3, space="PSUM") as wps_pool,
            ):
                hps = hps_pool.tile([128, SW], f32)
                ci = 0
                si = 0
                run = 0
                for s in range(cfg.n_sw):
                    for b in range(cfg.n_buckets):
                        bucket_ap = xb_d[b].ap()
                        for ncall in calls[run]:
                            nidx = ncall * 128
                            gout = eph.tile([128, cfg.gb, F], bf16, tag="gout")
                            nc.gpsimd.dma_gather(
                                out_ap=gout[:, :ncall, :],
                                in_ap=bucket_ap,
                                idxs_ap=idxw[:, si // 16:(si + nidx) // 16],
                                num_idxs=nidx, num_idxs_reg=nidx, elem_size=F,
                                single_packet=False)
                            ef = eph.tile([8, cfg.gb * 128], bf16, tag="ef")
                            nc.sync.dma_start(
                                out=ef[:, :nidx],
                                in_=efT_d.ap()[:, si:si + nidx])
                            oh = eph.tile([128, cfg.gb * GRP], bf16, tag="oh")
                            nc.sync.dma_start(
                                out=oh[:, :ncall * GRP],
                                in_=oh_d.ap()[:, ci * GRP:(ci + ncall) * GRP])
                            for q in range(0, ncall, 4):
                                bs = min(4, ncall - q)
                                pw4 = wps_pool.tile([128, 4, F], f32, tag="pw4")
                                for j in range(bs):
                                    nc.tensor.matmul(
                                        out=pw4[:, j, :],
                                        lhsT=ef[:, (q + j) * 128:
                                                (q + j + 1) * 128],
                                        rhs=ewT9[:], start=True, stop=True)
                                mp4 = mph.tile([128, 4, F], f32, tag="mp4")
                                nc.vector.tensor_add(
                                    out=mp4[:, :bs, :],
                                    in0=gout[:, q:q + bs, :],
                                    in1=pw4[:, :bs, :])
                                m4 = mph.tile([128, 4, F], bf16, tag="m4")
                                nc.scalar.activation(
                                    out=m4[:, :bs, :], in_=mp4[:, :bs, :],
                                    func=RELU)
                                for kk in range(bs):
                                    _s, _b, g, st, sp = sched[ci]
                                    assert (_s, _b) == (s, b)
                                    off = (g - s * cfg.gps) * GRP
                                    nc.tensor.matmul(
                                        out=hps[:, off:off + GRP],
                                        lhsT=m4[:, kk, :],
                                        rhs=oh[:, (q + kk) * GRP:
                                               (q + kk + 1) * GRP],
                                        start=st, stop=sp,
                                        skip_group_check=True)
                                    ci += 1
                            si += nidx
                        run += 1
                    w = SW if s < cfg.n_sw - 1 else cfg.last_w
                    nc.vector.tensor_add(
                        out=hT[:, s * SW:s * SW + w],
                        in0=hT[:, s * SW:s * SW + w], in1=hps[:, :w])
                assert ci == cfg.n_chunks and si == cfg.slots

            # ================= phase 3: node-side =================
            with (
                tc.tile_pool(name="nph", bufs=3) as nph,
                tc.tile_pool(name="nps", bufs=4, space="PSUM") as nps,
            ):
                NBLK = 8
                for blk in range(math.ceil(cfg.nsh_tiles / NBLK)):
                    t0 = blk * NBLK
                    nt = min(NBLK, cfg.nsh_tiles - t0)
                    xtile = nph.tile([128, NBLK, F], bf16, tag="xtile")
                    nc.sync.dma_start(
                        out=xtile[:, :nt, :],
                        in_=xb_d[0].ap()[t0 * 128:(t0 + nt) * 128, :].rearrange(
                            "(b p) f -> p b f", p=128))
                    ot = nph.tile([128, NBLK, F], f32, tag="ot")
                    for j in range(nt):
                        t = t0 + j
                        pt = nps.tile([128, F], f32, tag="pt")
                        nc.tensor.transpose(
                            out=pt[:], in_=hT[:, t * 128:(t + 1) * 128],
                            identity=ident[:])
                        s1 = nph.tile([128, F], f32, tag="s1")
                        nc.vector.tensor_scalar_mul(
                            out=s1[:], in0=pt[:], scalar1=disP[:, t:t + 1])
                        x1 = nph.tile([128, F], f32, tag="x1")
                        nc.vector.tensor_scalar_mul(
                            out=x1[:], in0=xtile[:, j, :],
                            scalar1=idisP[:, t:t + 1])
                        t1 = nph.tile([128, F], f32, tag="t1")
                        nc.vector.tensor_add(
                            out=t1[:], in0=x1[:], in1=rootB[:])
                        s2 = nph.tile([128, F], f32, tag="s2")
                        nc.scalar.activation(
                            out=s2[:], in_=t1[:], func=RELU,
                            scale=ivdP[:, t:t + 1])
                        nc.vector.tensor_add(out=ot[:, j, :], in0=s1[:],
                                             in1=s2[:])
                    nc.sync.dma_start(
                        out=out_d.ap()[t0 * 128:(t0 + nt) * 128, :].rearrange(
                            "(b p) f -> p b f", p=128),
                        in_=ot[:, :nt, :])
    nc.compile()
    return nc


# ------------------------------------------------------------- host prep ----
def compute_caps(cfg: Cfg, src, dst):
    caps = np.zeros((cfg.n_buckets, cfg.n_groups), dtype=np.int64)
    for c in range(cfg.cores):
        sel = np.nonzero(dst // cfg.NSH == c)[0]
        rsrc = (src[sel] - c * cfg.NSH) % cfg.N
        ed = dst[sel] - c * cfg.NSH
        key = (rsrc // cfg.bucket_sz) * cfg.n_groups + ed // cfg.GRP
        cnt = np.bincount(key, minlength=cfg.n_buckets * cfg.n_groups)
        need = np.ceil(cnt / 128).astype(np.int64).reshape(
            cfg.n_buckets, cfg.n_groups)
        caps = np.maximum(caps, need)
    return caps


def host_prep(cfg: Cfg, feat, edge_feat, src, dst, fc_w, edge_w, edge_b,
              root_emb):
    import ml_dtypes
    bf = ml_dtypes.bfloat16
    N, E, F = cfg.N, cfg.E, cfg.F
    deg = (np.bincount(dst, minlength=N) + 1.0).astype(np.float32)
    dis = deg ** -0.5

    featT_full = np.ascontiguousarray((feat * dis[:, None]).T).astype(np.float32)
    fcwT = np.ascontiguousarray(fc_w.T).astype(bf)
    ewT9 = np.zeros((8, F), dtype=np.float32)
    ewT9[:cfg.ED] = edge_w.T
    ewT9[7] = edge_b
    ewT9 = ewT9.astype(bf)
    rootB = np.tile(root_emb[0][None, :], (128, 1)).astype(np.float32)
    ident = np.eye(128, dtype=np.float32)

    sched = cfg.sched()
    # chunk start index per (s,b,g) cell in schedule order
    core_of = dst // cfg.NSH
    in_maps = []
    for c in range(cfg.cores):
        sel = np.nonzero(core_of == c)[0]
        rsrc = (src[sel] - c * cfg.NSH) % N
        ed = dst[sel] - c * cfg.NSH
        eb = rsrc // cfg.bucket_sz
        g = ed // cfg.GRP
        s_of_g = np.minimum(g // cfg.gps, cfg.n_sw - 1)
        comp = (s_of_g * cfg.n_buckets + eb) * cfg.n_groups + g
        order = np.lexsort((ed, comp))
        es, ed, comp = rsrc[order], ed[order], comp[order]
        eid = sel[order]

        slot_src = np.zeros(cfg.slots, dtype=np.int16)
        slot_rel = np.full(cfg.slots, -1, dtype=np.int64)
        slot_eid = np.full(cfg.slots, -1, dtype=np.int64)

        keys = [(s * cfg.n_buckets + b) * cfg.n_groups + gg
                for s in range(cfg.n_sw) for b in range(cfg.n_buckets)
                for gg in cfg.groups_of_sw(s)]
        seg_starts = np.searchsorted(comp, keys + [keys[-1] + 1])
        ci = 0
        seg = 0
        for s in range(cfg.n_sw):
            for b in range(cfg.n_buckets):
                for gg in cfg.groups_of_sw(s):
                    lo, hi = seg_starts[seg], seg_starts[seg + 1]
                    seg += 1
                    nseg = hi - lo
                    if math.ceil(nseg / 128) > cfg.caps[b, gg]:
                        raise RuntimeError(
                            f"overflow core {c} s {s} b {b} g {gg}: "
                            f"{math.ceil(nseg/128)} > {cfg.caps[b, gg]}")
                    slot0 = ci * 128
                    slot_src[slot0:slot0 + nseg] = (
                        es[lo:hi] - b * cfg.bucket_sz).astype(np.int16)
                    slot_rel[slot0:slot0 + nseg] = ed[lo:hi] - gg * cfg.GRP
                    slot_eid[slot0:slot0 + nseg] = eid[lo:hi]
                    ci += int(cfg.caps[b, gg])
        assert ci == cfg.n_chunks

        real = slot_eid >= 0
        efT = np.zeros((8, cfg.slots), dtype=bf)
        sdis = dis[src[slot_eid[real]]].astype(np.float32)
        efT[:cfg.ED, real] = (edge_feat[slot_eid[real]] *
                              sdis[:, None]).T.astype(bf)
        efT[7, real] = sdis.astype(bf)

        oh = np.zeros((128, cfg.n_chunks * cfg.GRP), dtype=bf)
        slots_idx = np.nonzero(real)[0]
        rows = slots_idx % 128
        cols = (slots_idx // 128) * cfg.GRP + slot_rel[slots_idx]
        oh[rows, cols] = 1.0

        idxw = np.zeros((16, cfg.slots // 16), dtype=np.int16)
        si = 0
        for sizes in cfg.call_layout():
            for nch in sizes:
                blkv = slot_src[si:si + nch * 128]
                idxw[:, si // 16:(si + nch * 128) // 16] = \
                    blkv.reshape(-1, 16).T
                si += nch * 128
        idxw = np.tile(idxw, (8, 1))

        nd = np.arange(cfg.NSHpad)
        gidx = np.minimum(c * cfg.NSH + nd, N - 1)
        disP = np.ascontiguousarray(dis[gidx].reshape(-1, 128).T)
        ivdP = np.ascontiguousarray((1.0 / deg[gidx]).reshape(-1, 128).T)
        idisP = np.ascontiguousarray((1.0 / dis[gidx]).reshape(-1, 128).T)

        featT = np.zeros((F, cfg.Npad), dtype=np.float32)
        featT[:, :N] = np.roll(featT_full, -c * cfg.NSH, axis=1)

        in_maps.append({
            "featT": featT.astype(bf), "fcwT": fcwT, "ewT9": ewT9,
            "rootB": rootB, "ident": ident,
            "efT": efT, "ohT": oh, "idxw": idxw,
            "disP": disP, "ivdP": ivdP, "idisP": idisP,
        })
    return in_maps


# ----------------------------------------------------------------- entry ----
def kernel(feat, edge_feat, src, dst, fc_w, edge_w, edge_b, root_emb,
           _trace=False, _cfg=None, **_kw):
    cfg = _cfg or CFG
    feat = np.asarray(feat); edge_feat = np.asarray(edge_feat)
    src = np.asarray(src); dst = np.asarray(dst)
    fc_w = np.asarray(fc_w); edge_w = np.asarray(edge_w)
    edge_b = np.asarray(edge_b); root_emb = np.asarray(root_emb)
    assert feat.shape == (cfg.N, cfg.F) and src.shape == (cfg.E,), \
        (feat.shape, src.shape)
    if cfg.caps is None:
        cfg.set_caps(compute_caps(cfg, src, dst))
    key = (id(cfg), tuple(cfg.caps.ravel()))
    if key not in _PROG_CACHE:
        _PROG_CACHE[key] = build_program(cfg)
    nc = _PROG_CACHE[key]
    in_maps = host_prep(
        cfg, feat, edge_feat, src, dst, fc_w, edge_w, edge_b, root_emb)
    res = bass_utils.run_bass_kernel_spmd(
        nc, in_maps, core_ids=list(range(cfg.cores)), trace=_trace)
    out = np.concatenate(
        [res.results[c]["out"][:cfg.NSH] for c in range(cfg.cores)], axis=0)
    kernel._last_results = res
    return out.astype(np.float32)
